# revision 1
# baseline (speedup 1.0000x reference)
"""DisMax loss first part: logits = -(|s|*d + mean_c(|s|*d)) / temp, where
d[b,c] = ||fn_b - pn_c|| / sqrt(2) = sqrt(1 - cos(f_b, p_c)) for l2-normalized rows.

Strategy: data-parallel over the batch across 8 NeuronCores. Each core:
  [1024, 512] features x [512, 10000] transposed prototypes -> [1024, 10000].
Prototypes are passed host-transposed (layout prep only) so the device GEMM
operands are already [d, .]; the device computes all numerics:
  - prototype class norms: DVE/GPSIMD squares -> PE ones-matmul column
    sums -> one ACT rsqrt per chunk pair -> PE partition-broadcast matmul ->
    fused DVE normalize+cast (f32 staging x inv-norm -> bf16 pnT);
  - feature row norms: DVE square+row-sum, ACT rsqrt, DVE scale-cast,
    PE transposes into fnT;
  - main GEMM: bf16, fp32 PSUM (2-bank tiles), [128 x 1000] chunks; ACT
    computes sqrt(1 - cos) out of PSUM with fused row-sum accumulation;
  - GPSIMD applies out = dist*c0 + rowsum*c1 (c0 = -|scale|/temp,
    c1 = c0/10000); 1 MB chunked DMAs stream the 40 MB result to HBM.
"""

import sys
import types

for _p in ("/opt/trn_rl_repo", "/root/.axon_site"):
    if _p not in sys.path:
        sys.path.insert(0, _p)

# The NTFF profiling hook module is absent from this image's antenv package;
# inject the ctypes-based equivalent so trace=True works when requested.
if "antenv.axon_hooks" not in sys.modules:
    try:
        import trn_agent_boot.trn_boot as _tb

        _hook = _tb._ntff_profile_via_ctypes("/opt/axon/libaxon_pjrt.so")
        _m = types.ModuleType("antenv.axon_hooks")
        _m.get_axon_ntff_profile_hook = lambda: _hook
        sys.modules["antenv.axon_hooks"] = _m
    except Exception:
        pass

import numpy as np

import concourse.bacc as bacc
import concourse.tile as tile
import concourse.mybir as mybir
from concourse.bass_utils import run_bass_kernel_spmd

F32 = mybir.dt.float32
BF16 = mybir.dt.bfloat16
ALU = mybir.AluOpType
ACTF = mybir.ActivationFunctionType

N_CORES = 8
B, C, D = 8192, 10000, 512
BPC = B // N_CORES          # 1024 batch rows per core
NB = BPC // 128             # 8 batch tiles
ND = D // 128               # 4 contraction tiles
CCH = 500                   # matmul free-dim chunk (fits 1 PSUM bank in f32)
BCH = 1000                  # ACT/sqrt + prototype-prep chunk
NBCH = C // BCH             # 10 chunks
P2 = 2000                   # pass-2 / store chunk (1 MB DMA)
NP2 = C // P2               # 5
FG = 1                      # feature tiles per staged load (256 KB DMA)


def build_nc():
    nc = bacc.Bacc("TRN2", target_bir_lowering=False, debug=False,
                   num_devices=N_CORES)
    f_h = nc.dram_tensor("f", [BPC, D], F32, kind="ExternalInput")
    pt_h = nc.dram_tensor("pt", [D, C], F32, kind="ExternalInput")
    s_h = nc.dram_tensor("s", [1, 2], F32, kind="ExternalInput")
    o_h = nc.dram_tensor("o", [BPC, C], F32, kind="ExternalOutput")

    from contextlib import ExitStack

    with tile.TileContext(nc) as tc:
        with ExitStack() as stack:
            ep = stack.enter_context
            const_pool = ep(tc.tile_pool(name="const", bufs=1))
            persist_pool = ep(tc.tile_pool(name="persist", bufs=1))
            fstage_pool = ep(tc.tile_pool(name="fstage", bufs=2))
            pstage_pool = ep(tc.tile_pool(name="pstage", bufs=4))
            rows_pool = ep(tc.tile_pool(name="rows", bufs=1))
            bf_pool = ep(tc.tile_pool(name="bfc", bufs=2))
            sq_pool = ep(tc.tile_pool(name="sq", bufs=4))
            norm_pool = ep(tc.tile_pool(name="norms", bufs=2))
            invb_pool = ep(tc.tile_pool(name="invb", bufs=2))
            dist_pool = ep(tc.tile_pool(name="dist", bufs=6))
            rs_pool = ep(tc.tile_pool(name="rs", bufs=2))
            psum_c_pool = ep(tc.tile_pool(name="ps_c", bufs=3, space="PSUM"))
            psum_m_pool = ep(tc.tile_pool(name="ps_m", bufs=2, space="PSUM"))
            # persistent bf16 transposed operands
            pnT = persist_pool.tile([128, ND, C], BF16, tag="pnT")     # 78 KB/p
            fnT = persist_pool.tile([128, ND, BPC], BF16, tag="fnT")   # 8 KB/p
            cb = persist_pool.tile([128, 2], F32, tag="cb")            # c0, c1

            ones_f = const_pool.tile([1, 128], F32, tag="ones_f")
            nc.vector.memset(ones_f[:, :], 1.0)
            ones_b = const_pool.tile([128, 1], BF16, tag="ones_b")
            nc.vector.memset(ones_b[:, :], 1.0)
            from concourse import masks

            ident = const_pool.tile([128, 128], BF16, tag="ident")
            masks.make_identity(nc, ident[:, :])

            # ---- scalar params: c0 = -|ds|/temp, c1 = c0/C ----------------
            stile = const_pool.tile([1, 2], F32, tag="stile")
            nc.sync.dma_start(stile[:, :], s_h[:, :])
            cv = const_pool.tile([1, 2], F32, tag="cvals")
            tmp = const_pool.tile([1, 2], F32, tag="scaltmp")
            nc.scalar.activation(tmp[:, 0:1], stile[:, 0:1], ACTF.Abs)
            nc.vector.reciprocal(tmp[:, 1:2], stile[:, 1:2])
            nc.vector.scalar_tensor_tensor(cv[:, 0:1], tmp[:, 0:1], -1.0,
                                           tmp[:, 1:2], op0=ALU.mult,
                                           op1=ALU.mult)
            nc.vector.tensor_scalar(cv[:, 1:2], cv[:, 0:1], 1.0 / C, None,
                                    op0=ALU.mult)
            ps_b = psum_m_pool.tile([128, CCH], F32, tag="m")
            nc.tensor.matmul(ps_b[:, :2], ones_f[:, :], cv[:, :], start=True,
                             stop=True)
            nc.vector.tensor_copy(cb[:, :], ps_b[:, :2])

            # ---- feature prep ---------------------------------------------
            f_r = f_h[:, :].rearrange("(g t p) d -> g p t d", p=128, t=FG)
            for g in range(NB // FG):
                fst = fstage_pool.tile([128, FG, D], F32, tag="fst")
                nc.sync.dma_start(fst[:, :, :], f_r[g])
                fss = norm_pool.tile([128, FG], F32, tag="fss")
                finv = norm_pool.tile([128, FG], F32, tag="finv")
                for t in range(FG):
                    fsq = bf_pool.tile([128, D], BF16, tag="bfc", name=f"fsq_{g}_{t}")
                    nc.vector.scalar_tensor_tensor(
                        fsq[:, :], fst[:, t, :], 1.0, fst[:, t, :],
                        op0=ALU.mult, op1=ALU.mult,
                        accum_out=fss[:, t:t + 1])
                nc.scalar.activation(finv[:, :], fss[:, :],
                                     ACTF.Abs_reciprocal_sqrt)
                for t in range(FG):
                    i = g * FG + t
                    fbf = bf_pool.tile([128, D], BF16, tag="bfc")
                    nc.vector.tensor_scalar(fbf[:, :], fst[:, t, :],
                                            finv[:, t:t + 1], None,
                                            op0=ALU.mult)
                    ps_t0 = psum_m_pool.tile([128, CCH], F32, tag="m", name="ps_t0")
                    ps_t = ps_t0[:, :].bitcast(BF16)[:, :ND * 128].rearrange("p (d c) -> p d c", d=ND)
                    for d in range(ND):
                        nc.tensor.transpose(ps_t[:, d, :],
                                            fbf[:, d * 128:(d + 1) * 128],
                                            ident[:, :])
                    nc.vector.tensor_copy(
                        fnT[:, :, i * 128:(i + 1) * 128], ps_t[:, :, :])

            # ---- prototype prep (host-transposed pT in DRAM) ---------------
            # processed in pairs of 1000-class chunks so the ACT rsqrt (and
            # its activation-table load) runs once per pair
            pt_r = pt_h[:, :].rearrange("(t p) c -> p t c", p=128)
            for pp in range(NBCH // 2):
                pair_psts = []
                srow = rows_pool.tile([1, 2 * BCH], F32, tag="srow")
                irow = srow
                for ci in range(2):
                    cc = 2 * pp + ci
                    c0, c1 = cc * BCH, (cc + 1) * BCH
                    psts = []
                    sqs = []
                    for h in range(2):
                        pst = pstage_pool.tile([128, 2, BCH], F32, tag="pstg",
                                               name=f"pst_{cc}_{h}")
                        psts.append(pst)
                        sqh = sq_pool.tile([128, 2, BCH], BF16, tag="sq",
                                           name=f"sq_{cc}_{h}")
                        sqs.append(sqh)
                        nc.sync.dma_start(pst[:, :, :],
                                          pt_r[:, 2 * h:2 * h + 2, c0:c1])
                        for hh in range(2):
                            # squares for the class-norm column sums, split
                            # DVE/GPSIMD (not ACT: keeps the table stable)
                            sqeng = (nc.gpsimd if (h == 0 and hh == 0)
                                     else nc.vector)
                            sqeng.tensor_tensor(sqh[:, hh, :], pst[:, hh, :],
                                                pst[:, hh, :], op=ALU.mult)
                    pair_psts.append(psts)
                    for sub in range(2):
                        ss0 = psum_m_pool.tile([128, CCH], F32, tag="m",
                                               name="ss0")
                        ss = ss0[:1, :]
                        for d in range(ND):
                            nc.tensor.matmul(
                                ss[:, :], ones_b[:, :],
                                sqs[d // 2][:, d % 2,
                                            sub * CCH:(sub + 1) * CCH],
                                start=(d == 0), stop=(d == ND - 1))
                        off = ci * BCH + sub * CCH
                        nc.vector.tensor_copy(srow[:, off:off + CCH],
                                              ss[:, :])
                # 1/norm in a single ACT op per pair
                nc.scalar.activation(irow[:, :], srow[:, :],
                                     ACTF.Abs_reciprocal_sqrt)
                for ci in range(2):
                    cc = 2 * pp + ci
                    c0, c1 = cc * BCH, (cc + 1) * BCH
                    ib = invb_pool.tile([128, BCH], F32, tag="invb")
                    for sub in range(2):
                        bc = psum_m_pool.tile([128, CCH], F32, tag="m")
                        off = ci * BCH + sub * CCH
                        nc.tensor.matmul(bc[:, :], ones_f[:, :],
                                         irow[:, off:off + CCH],
                                         start=True, stop=True)
                        nc.vector.tensor_copy(
                            ib[:, sub * CCH:(sub + 1) * CCH], bc[:, :])
                    # fused normalize + cast: pnT = pst * (1/||p_c||)
                    for d in range(ND):
                        nc.vector.tensor_tensor(
                            pnT[:, d, c0:c1],
                            pair_psts[ci][d // 2][:, d % 2, :],
                            ib[:, :], op=ALU.mult)

            # ---- main loop -------------------------------------------------
            for i in range(NB):
                rs = rs_pool.tile([128, NBCH], F32, tag="rs")
                dqs = []
                for q in range(NP2):
                    dq = dist_pool.tile([128, P2], F32, tag="dist")
                    dqs.append(dq)
                    for k in range(P2 // BCH):
                        bc = q * (P2 // BCH) + k           # 1000-chunk index
                        pc = psum_c_pool.tile([128, 2, 512], F32, tag="pc")
                        for h in range(2):
                            c0 = bc * BCH + h * CCH
                            for d in range(ND):
                                nc.tensor.matmul(
                                    pc[:, h, :CCH],
                                    fnT[:, d, i * 128:(i + 1) * 128],
                                    pnT[:, d, c0:c0 + CCH],
                                    start=(d == 0), stop=(d == ND - 1))
                        # dist = sqrt(1 - cos); accum_out = row-chunk sum
                        dv = dq[:, k * BCH:(k + 1) * BCH].rearrange(
                            "p (h c) -> p h c", h=2)
                        nc.scalar.activation(
                            dv, pc[:, :, :CCH],
                            ACTF.Sqrt, bias=1.0, scale=-1.0,
                            accum_out=rs[:, bc:bc + 1])
                rsum = norm_pool.tile([128, 1], F32, tag="rsum")
                bvec = norm_pool.tile([128, 1], F32, tag="bvec")
                nc.vector.reduce_sum(rsum[:, :], rs[:, :],
                                     axis=mybir.AxisListType.X)
                nc.vector.tensor_scalar(bvec[:, :], rsum[:, :], cb[:, 1:2],
                                        None, op0=ALU.mult)
                for q in range(NP2):
                    ob0 = pstage_pool.tile([128, 2, BCH], F32, tag="pstg",
                                           name=f"ob_{i}_{q}")
                    ob = ob0[:, :, :].rearrange("p a b -> p (a b)")
                    nc.gpsimd.tensor_scalar(ob, dqs[q][:, :],
                                            cb[:, 0:1], bvec[:, 0:1],
                                            op0=ALU.mult, op1=ALU.add)
                    nc.sync.dma_start(
                        o_h[i * 128:(i + 1) * 128, q * P2:(q + 1) * P2],
                        ob)

    nc.compile()
    return nc


_CACHE = {}


def _get_nc():
    if "nc" not in _CACHE:
        _CACHE["nc"] = build_nc()
    return _CACHE["nc"]


def make_in_maps(features, prototypes, distance_scale, temperature):
    f = np.ascontiguousarray(np.asarray(features, dtype=np.float32))
    pt = np.ascontiguousarray(np.asarray(prototypes, dtype=np.float32).T)
    s = np.array([[np.float32(np.asarray(distance_scale).reshape(-1)[0]),
                   np.float32(np.asarray(temperature).reshape(-1)[0])]],
                 dtype=np.float32)
    return [
        {"f": f[i * BPC:(i + 1) * BPC], "pt": pt, "s": s}
        for i in range(N_CORES)
    ]


def run(features, prototypes, distance_scale, temperature, **kwargs):
    nc = _get_nc()
    in_maps = make_in_maps(features, prototypes, distance_scale, temperature)
    res = run_bass_kernel_spmd(nc, in_maps, core_ids=list(range(N_CORES)),
                               **kwargs)
    out = np.concatenate([res.results[i]["o"] for i in range(N_CORES)], axis=0)
    return out, res


def kernel(features, prototypes, distance_scale, temperature):
    out, _ = run(features, prototypes, distance_scale, temperature)
    return out



# revision 4
# speedup vs baseline: 1.2062x; 1.2062x over previous
"""DisMax loss first part: logits = -(|s|*d + mean_c(|s|*d)) / temp, where
d[b,c] = ||fn_b - pn_c|| / sqrt(2) = sqrt(1 - cos(f_b, p_c)) for l2-normalized rows.

Strategy: data-parallel over the batch across 8 NeuronCores. Each core:
  [1024, 512] features x [512, 10000] transposed prototypes -> [1024, 10000].
Both operands arrive host-transposed and host-cast to bf16 (layout/precision
prep only); all numerics run on device:
  - feature row norms: DVE square+accum on the batch-major copy, one ACT
    abs-rsqrt -> invf [128, 8]; features themselves stay RAW (negated on
    host) and invf folds into the main ACT as its per-partition scale;
  - prototype class norms: DVE/GPSIMD squares -> PE ones-matmul column
    sums -> chunked ACT abs-rsqrt (bf16 row) -> PE partition-broadcast ->
    one DVE normalize multiply into the persistent bf16 pn operand;
  - main GEMM: bf16, fp32 PSUM, [128 x 500] chunks; ACT computes
    dist = sqrt(1 + G*invf) straight out of PSUM (G = -f.pn) with fused
    row-sum accumulation, writing bf16;
  - DVE applies out = dist*c0 + rowsum*c1 (c0 = -|scale|/temp,
    c1 = c0/10000) at 4x bf16 rate; 1.25 MB DMAs stream the bf16 result;
  - host upcasts the bf16 logits to f32 (within the 2e-2 tolerance).
"""

import sys
import types

for _p in ("/opt/trn_rl_repo", "/root/.axon_site"):
    if _p not in sys.path:
        sys.path.insert(0, _p)

# The NTFF profiling hook module is absent from this image's antenv package;
# inject the ctypes-based equivalent so trace=True works when requested.
if "antenv.axon_hooks" not in sys.modules:
    try:
        import trn_agent_boot.trn_boot as _tb

        _hook = _tb._ntff_profile_via_ctypes("/opt/axon/libaxon_pjrt.so")
        _m = types.ModuleType("antenv.axon_hooks")
        _m.get_axon_ntff_profile_hook = lambda: _hook
        sys.modules["antenv.axon_hooks"] = _m
    except Exception:
        pass

import ml_dtypes
import numpy as np

import concourse.bacc as bacc
import concourse.tile as tile
import concourse.mybir as mybir
from concourse.bass_utils import run_bass_kernel_spmd

F32 = mybir.dt.float32
BF16 = mybir.dt.bfloat16
ALU = mybir.AluOpType
ACTF = mybir.ActivationFunctionType

N_CORES = 8
B, C, D = 8192, 10000, 512
BPC = B // N_CORES          # 1024 batch rows per core
NB = BPC // 128             # 8 batch tiles
ND = D // 128               # 4 contraction tiles
PCH = 1000                  # prototype-prep chunk (columns)
NPCH = C // PCH             # 10
G1 = 1000                   # main-loop ACT group (2 psum banks)
NG = C // G1                # 10
OUT = 5000                  # output store chunk (1.25 MB bf16)
NOUT = C // OUT             # 2
# which prototype-prep square chunks run on gpsimd (rest on DVE)
SQ_GPS = {4, 9}


def build_nc():
    nc = bacc.Bacc("TRN2", target_bir_lowering=False, debug=False,
                   num_devices=N_CORES)
    ft_h = nc.dram_tensor("ft", [D, BPC], BF16, kind="ExternalInput")
    fb_h = nc.dram_tensor("fb", [BPC, D], BF16, kind="ExternalInput")
    pt_h = nc.dram_tensor("pt", [D, C], BF16, kind="ExternalInput")
    s_h = nc.dram_tensor("s", [1, 2], F32, kind="ExternalInput")
    o_h = nc.dram_tensor("o", [BPC, C], BF16, kind="ExternalOutput")

    from contextlib import ExitStack

    with tile.TileContext(nc) as tc:
        with ExitStack() as stack:
            ep = stack.enter_context
            const_pool = ep(tc.tile_pool(name="const", bufs=1))
            persist_pool = ep(tc.tile_pool(name="persist", bufs=1))
            pstage_pool = ep(tc.tile_pool(name="pstage", bufs=2))
            sq_pool = ep(tc.tile_pool(name="sq", bufs=2))
            dq_pool = ep(tc.tile_pool(name="dq", bufs=2))
            ob_pool = ep(tc.tile_pool(name="ob", bufs=2))
            small_pool = ep(tc.tile_pool(name="small", bufs=2))
            ps_c = ep(tc.tile_pool(name="ps_c", bufs=3, space="PSUM"))
            ps_n = ep(tc.tile_pool(name="ps_n", bufs=1, space="PSUM"))

            # persistent operands
            pn = persist_pool.tile([128, ND, C], BF16, tag="pn")      # 78 KB/p
            fT = persist_pool.tile([128, ND, BPC], BF16, tag="fT")    # 8 KB/p
            invpb = persist_pool.tile([128, C], BF16, tag="invpb")    # 20 KB/p
            invf = persist_pool.tile([128, NB], F32, tag="invf")
            cb = persist_pool.tile([128, 2], F32, tag="cb")           # c0, c1

            ones_b = const_pool.tile([128, 1], BF16, tag="ones_b")
            nc.vector.memset(ones_b[:, :], 1.0)
            ones_r = const_pool.tile([1, 128], BF16, tag="ones_r")
            nc.vector.memset(ones_r[:, :], 1.0)
            ones_rf = const_pool.tile([1, 128], F32, tag="ones_rf")
            nc.vector.memset(ones_rf[:, :], 1.0)
            one_f = const_pool.tile([1, 1], F32, tag="one_f")
            nc.vector.memset(one_f[:, :], 1.0)

            # ---- scalar params: c0 = -|ds|/temp, c1 = c0/C ----------------
            stile = const_pool.tile([1, 2], F32, tag="stile")
            nc.sync.dma_start(stile[:, :], s_h[:, :])
            cv = const_pool.tile([1, 2], F32, tag="cvals")
            tmp = const_pool.tile([1, 2], F32, tag="scaltmp")
            nc.scalar.activation(tmp[:, 0:1], stile[:, 0:1], ACTF.Abs)
            nc.vector.reciprocal(tmp[:, 1:2], stile[:, 1:2])
            nc.vector.scalar_tensor_tensor(cv[:, 0:1], tmp[:, 0:1], -1.0,
                                           tmp[:, 1:2], op0=ALU.mult,
                                           op1=ALU.mult)
            nc.vector.tensor_scalar(cv[:, 1:2], cv[:, 0:1], 1.0 / C, None,
                                    op0=ALU.mult)
            ps_s = ps_c.tile([128, 2, 512], F32, tag="pc", name="ps_s")
            nc.tensor.matmul(ps_s[:, 0, :2], ones_rf[:, :], cv[:, :],
                             start=True, stop=True)
            nc.vector.tensor_copy(cb[:, :], ps_s[:, 0, :2])

            # ---- feature prep ---------------------------------------------
            # transposed raw (negated) features for the GEMM
            ft_r = ft_h[:, :].rearrange("(t p) b -> p t b", p=128)
            nc.sync.dma_start(fT[:, :, :], ft_r)
            # batch-major copy solely for the row norms
            fb_r = fb_h[:, :].rearrange("(t p) d -> p t d", p=128)
            fst = pstage_pool.tile([128, NB, D], BF16, tag="pstg", name="fst")
            nc.sync.dma_start(fst[:, :, :], fb_r)
            fss = small_pool.tile([128, NB], F32, tag="fss", bufs=1)
            for t in range(NB):
                fsq = sq_pool.tile([128, ND, PCH], BF16, tag="sq",
                                   name=f"fsq_{t}")
                fsqv = fsq[:, :, :].rearrange("p a b -> p (a b)")
                nc.vector.scalar_tensor_tensor(
                    fsqv[:, :D], fst[:, t, :], 1.0, fst[:, t, :],
                    op0=ALU.mult, op1=ALU.mult,
                    accum_out=fss[:, t:t + 1])
            nc.scalar.activation(invf[:, :], fss[:, :],
                                 ACTF.Abs_reciprocal_sqrt)

            # ---- prototype prep (host-transposed bf16 pT in DRAM) ----------
            pt_r = pt_h[:, :].rearrange("(t p) c -> p t c", p=128)
            for c in range(NPCH):
                c0, c1 = c * PCH, (c + 1) * PCH
                pst = pstage_pool.tile([128, ND, PCH], BF16, tag="pstg",
                                       name=f"pst_{c}")
                nc.sync.dma_start(pst[:, :, :], pt_r[:, :, c0:c1])
                sq = sq_pool.tile([128, ND, PCH], BF16, tag="sq",
                                  name=f"sq_{c}")
                sqeng = nc.gpsimd if c in SQ_GPS else nc.vector
                sqeng.tensor_tensor(
                    sq[:, :, :].rearrange("p a b -> p (a b)"),
                    pst[:, :, :].rearrange("p a b -> p (a b)"),
                    pst[:, :, :].rearrange("p a b -> p (a b)"),
                    op=ALU.mult)
                # class-norm column sums via ones-matmul, 2 x 500 halves
                psn = ps_n.tile([128, 2, 512], F32, tag="psn")
                for h in range(2):
                    for d in range(ND):
                        nc.tensor.matmul(
                            psn[0:1, h, :500], ones_b[:, :],
                            sq[:, d, h * 500:(h + 1) * 500],
                            start=(d == 0), stop=(d == ND - 1))
                # 1/||p|| straight into partition 0 of the broadcast buffer
                nc.scalar.activation(
                    invpb[0:1, c0:c1].rearrange("p (h x) -> p h x", h=2),
                    psn[0:1, :, :500], ACTF.Abs_reciprocal_sqrt)
                # broadcast to all 128 partitions (bf16 matmul) + copy out
                for h in range(2):
                    q0 = c0 + h * 500
                    psb = ps_c.tile([128, 2, 512], F32, tag="pc",
                                    name=f"psb_{c}_{h}")
                    nc.tensor.matmul(psb[:, 0, :500], ones_r[:, :],
                                     invpb[0:1, q0:q0 + 500],
                                     start=True, stop=True)
                    nc.scalar.copy(invpb[:, q0:q0 + 500], psb[:, 0, :500])
                # normalize: pn = pst * invp  (bf16, per d-tile)
                for d in range(ND):
                    nc.vector.tensor_tensor(pn[:, d, c0:c1],
                                            pst[:, d, :],
                                            invpb[:, c0:c1], op=ALU.mult)

            # ---- main loop -------------------------------------------------
            for i in range(NB):
                rs = small_pool.tile([128, NG], F32, tag="rs")
                dq = dq_pool.tile([128, C], BF16, tag="dq")
                for g in range(NG):
                    pc = ps_c.tile([128, 2, 512], F32, tag="pc")
                    for h in range(2):
                        c0 = g * G1 + h * 500
                        for d in range(ND):
                            nc.tensor.matmul(
                                pc[:, h, :500],
                                fT[:, d, i * 128:(i + 1) * 128],
                                pn[:, d, c0:c0 + 500],
                                start=(d == 0), stop=(d == ND - 1))
                    # dist = sqrt(1 + G*invf); fused row-chunk sum
                    dv = dq[:, g * G1:(g + 1) * G1].rearrange(
                        "p (h x) -> p h x", h=2)
                    nc.scalar.activation(
                        dv, pc[:, :, :500], ACTF.Sqrt,
                        bias=1.0, scale=invf[:, i:i + 1],
                        accum_out=rs[:, g:g + 1])
                rsum = small_pool.tile([128, 1], F32, tag="rsum")
                bvec = small_pool.tile([128, 1], F32, tag="bvec")
                nc.vector.reduce_sum(rsum[:, :], rs[:, :],
                                     axis=mybir.AxisListType.X)
                nc.vector.tensor_scalar(bvec[:, :], rsum[:, :], cb[:, 1:2],
                                        None, op0=ALU.mult)
                for q in range(NOUT):
                    ob = ob_pool.tile([128, OUT], BF16, tag="ob")
                    nc.vector.tensor_scalar(ob[:, :],
                                            dq[:, q * OUT:(q + 1) * OUT],
                                            cb[:, 0:1], bvec[:, 0:1],
                                            op0=ALU.mult, op1=ALU.add)
                    nc.sync.dma_start(
                        o_h[i * 128:(i + 1) * 128, q * OUT:(q + 1) * OUT],
                        ob[:, :])

    nc.compile()
    return nc


_CACHE = {}


def _get_nc():
    if "nc" not in _CACHE:
        _CACHE["nc"] = build_nc()
    return _CACHE["nc"]


def make_in_maps(features, prototypes, distance_scale, temperature):
    f = np.asarray(features, dtype=np.float32)
    # negated so ACT's positive per-partition scale yields 1 - cos
    fneg = (-f).astype(ml_dtypes.bfloat16)
    pt = np.ascontiguousarray(
        np.asarray(prototypes, dtype=np.float32).T).astype(ml_dtypes.bfloat16)
    s = np.array([[np.float32(np.asarray(distance_scale).reshape(-1)[0]),
                   np.float32(np.asarray(temperature).reshape(-1)[0])]],
                 dtype=np.float32)
    in_maps = []
    for i in range(N_CORES):
        fi = fneg[i * BPC:(i + 1) * BPC]
        in_maps.append({
            "ft": np.ascontiguousarray(fi.T),
            "fb": np.ascontiguousarray(fi),
            "pt": pt,
            "s": s,
        })
    return in_maps


def run(features, prototypes, distance_scale, temperature, **kwargs):
    nc = _get_nc()
    in_maps = make_in_maps(features, prototypes, distance_scale, temperature)
    res = run_bass_kernel_spmd(nc, in_maps, core_ids=list(range(N_CORES)),
                               **kwargs)
    out = np.concatenate(
        [np.asarray(res.results[i]["o"]) for i in range(N_CORES)],
        axis=0).astype(np.float32)
    return out, res


def kernel(features, prototypes, distance_scale, temperature):
    out, _ = run(features, prototypes, distance_scale, temperature)
    return out


# revision 6
# speedup vs baseline: 1.2163x; 1.0084x over previous
"""DisMax loss first part: logits = -(|s|*d + mean_c(|s|*d)) / temp, where
d[b,c] = ||fn_b - pn_c|| / sqrt(2) = sqrt(1 - cos(f_b, p_c)) for l2-normalized rows.

Strategy: data-parallel over the batch across 8 NeuronCores. Each core:
  [1024, 512] features x [512, 10000] transposed prototypes -> [1024, 10000].
Both operands arrive host-transposed and host-cast to bf16 (layout/precision
prep only); all numerics run on device:
  - feature row norms: DVE square+accum on the batch-major copy, one ACT
    abs-rsqrt -> invf [128, 8]; features themselves stay RAW (negated on
    host) and invf folds into the main ACT as its per-partition scale;
  - prototype class norms: DVE/GPSIMD squares -> PE ones-matmul column
    sums -> chunked ACT abs-rsqrt (bf16 row) -> PE partition-broadcast ->
    in-place DVE normalize of the persistent bf16 pn operand;
  - main GEMM: bf16, fp32 PSUM, [128 x 500] chunks into 4-bank groups; ACT
    computes dist = sqrt(1 + G*invf) straight out of PSUM (G = -f.pn) with
    fused row-sum accumulation, writing bf16;
  - DVE applies out = dist*c0 + rowsum*c1 (c0 = -|scale|/temp,
    c1 = c0/10000) at 4x bf16 rate; 1.25 MB DMAs stream the bf16 result;
  - host upcasts the bf16 logits to f32 (within the 2e-2 tolerance).
"""

import sys
import types

for _p in ("/opt/trn_rl_repo", "/root/.axon_site"):
    if _p not in sys.path:
        sys.path.insert(0, _p)

# The NTFF profiling hook module is absent from this image's antenv package;
# inject the ctypes-based equivalent so trace=True works when requested.
if "antenv.axon_hooks" not in sys.modules:
    try:
        import trn_agent_boot.trn_boot as _tb

        _hook = _tb._ntff_profile_via_ctypes("/opt/axon/libaxon_pjrt.so")
        _m = types.ModuleType("antenv.axon_hooks")
        _m.get_axon_ntff_profile_hook = lambda: _hook
        sys.modules["antenv.axon_hooks"] = _m
    except Exception:
        pass

import ml_dtypes
import numpy as np

import concourse.bacc as bacc
import concourse.tile as tile
import concourse.mybir as mybir
from concourse.bass_utils import run_bass_kernel_spmd

F32 = mybir.dt.float32
BF16 = mybir.dt.bfloat16
ALU = mybir.AluOpType
ACTF = mybir.ActivationFunctionType

N_CORES = 8
B, C, D = 8192, 10000, 512
BPC = B // N_CORES          # 1024 batch rows per core
NB = BPC // 128             # 8 batch tiles
ND = D // 128               # 4 contraction tiles
PCH = 1000                  # prototype-prep chunk (columns)
NPCH = C // PCH             # 10
G1 = 2000                   # main-loop ACT group (4 psum banks)
NG = C // G1                # 5
OUT = 5000                  # output store chunk (1.25 MB bf16)
NOUT = C // OUT             # 2
# which prototype-prep square chunks run on gpsimd (rest on DVE)
SQ_GPS = {4, 9}


def build_nc():
    nc = bacc.Bacc("TRN2", target_bir_lowering=False, debug=False,
                   num_devices=N_CORES)
    ft_h = nc.dram_tensor("ft", [D, BPC], BF16, kind="ExternalInput")
    fb_h = nc.dram_tensor("fb", [BPC, D], BF16, kind="ExternalInput")
    pt_h = nc.dram_tensor("pt", [D, C], BF16, kind="ExternalInput")
    s_h = nc.dram_tensor("s", [1, 2], F32, kind="ExternalInput")
    o_h = nc.dram_tensor("o", [BPC, C], BF16, kind="ExternalOutput")

    from contextlib import ExitStack

    with tile.TileContext(nc) as tc:
        with ExitStack() as stack:
            ep = stack.enter_context
            const_pool = ep(tc.tile_pool(name="const", bufs=1))
            persist_pool = ep(tc.tile_pool(name="persist", bufs=1))
            fst_pool = ep(tc.tile_pool(name="fst", bufs=1))
            sq_pool = ep(tc.tile_pool(name="sq", bufs=3))
            dq_pool = ep(tc.tile_pool(name="dq", bufs=2))
            ob_pool = ep(tc.tile_pool(name="ob", bufs=2))
            small_pool = ep(tc.tile_pool(name="small", bufs=2))
            ps_c = ep(tc.tile_pool(name="ps_c", bufs=2, space="PSUM"))

            # persistent operands
            pn = persist_pool.tile([128, ND, C], BF16, tag="pn")      # 78 KB/p
            fT = persist_pool.tile([128, ND, BPC], BF16, tag="fT")    # 8 KB/p
            invpb = persist_pool.tile([128, C], BF16, tag="invpb")    # 20 KB/p
            invf = persist_pool.tile([128, NB], F32, tag="invf")
            cb = persist_pool.tile([128, 2], F32, tag="cb")           # c0, c1

            ones_b = const_pool.tile([128, 1], BF16, tag="ones_b")
            nc.vector.memset(ones_b[:, :], 1.0)
            ones_r = const_pool.tile([1, 128], BF16, tag="ones_r")
            nc.vector.memset(ones_r[:, :], 1.0)
            ones_rf = const_pool.tile([1, 128], F32, tag="ones_rf")
            nc.vector.memset(ones_rf[:, :], 1.0)

            # ---- input DMAs (front-loaded) --------------------------------
            stile = const_pool.tile([1, 2], F32, tag="stile")
            nc.sync.dma_start(stile[:, :], s_h[:, :])
            ft_r = ft_h[:, :].rearrange("(t p) b -> p t b", p=128)
            nc.sync.dma_start(fT[:, :, :], ft_r)
            fb_r = fb_h[:, :].rearrange("(t p) d -> p t d", p=128)
            fst = fst_pool.tile([128, NB, D], BF16, tag="fst")
            nc.sync.dma_start(fst[:, :, :], fb_r)
            pt_r = pt_h[:, :].rearrange("(t p) c -> p t c", p=128)
            for c in range(NPCH):
                nc.sync.dma_start(pn[:, :, c * PCH:(c + 1) * PCH],
                                  pt_r[:, :, c * PCH:(c + 1) * PCH])

            # ---- scalar params: c0 = -|ds|/temp, c1 = c0/C ----------------
            cv = const_pool.tile([1, 2], F32, tag="cvals")
            tmp = const_pool.tile([1, 2], F32, tag="scaltmp")
            nc.scalar.activation(tmp[:, 0:1], stile[:, 0:1], ACTF.Abs)
            nc.vector.reciprocal(tmp[:, 1:2], stile[:, 1:2])
            nc.vector.scalar_tensor_tensor(cv[:, 0:1], tmp[:, 0:1], -1.0,
                                           tmp[:, 1:2], op0=ALU.mult,
                                           op1=ALU.mult)
            nc.vector.tensor_scalar(cv[:, 1:2], cv[:, 0:1], 1.0 / C, None,
                                    op0=ALU.mult)
            ps_s = ps_c.tile([128, 4, 512], F32, tag="pc", name="ps_s")
            nc.tensor.matmul(ps_s[:, 0, :2], ones_rf[:, :], cv[:, :],
                             start=True, stop=True)
            nc.vector.tensor_copy(cb[:, :], ps_s[:, 0, :2])

            # ---- feature norms --------------------------------------------
            fss = small_pool.tile([128, NB], F32, tag="fss", bufs=1)
            for t in range(NB):
                fsq = sq_pool.tile([128, ND, PCH], BF16, tag="sq",
                                   name=f"fsq_{t}")
                fsqv = fsq[:, :, :].rearrange("p a b -> p (a b)")
                nc.vector.scalar_tensor_tensor(
                    fsqv[:, :D], fst[:, t, :], 1.0, fst[:, t, :],
                    op0=ALU.mult, op1=ALU.mult,
                    accum_out=fss[:, t:t + 1])
            nc.scalar.activation(invf[:, :], fss[:, :],
                                 ACTF.Abs_reciprocal_sqrt)

            # ---- prototype prep: norms + in-place normalize ----------------
            for c in range(NPCH):
                c0, c1 = c * PCH, (c + 1) * PCH
                sq = sq_pool.tile([128, ND, PCH], BF16, tag="sq",
                                  name=f"sq_{c}")
                sqeng = nc.gpsimd if c in SQ_GPS else nc.vector
                sqeng.tensor_tensor(sq[:, :, :], pn[:, :, c0:c1],
                                    pn[:, :, c0:c1], op=ALU.mult)
                # one 4-bank psum tile per chunk: banks 2-3 column sums,
                # banks 0-1 partition-broadcast
                psq = ps_c.tile([128, 4, 512], F32, tag="pc",
                                name=f"psq_{c}")
                for h in range(2):
                    for d in range(ND):
                        nc.tensor.matmul(
                            psq[0:1, 2 + h, :500], ones_b[:, :],
                            sq[:, d, h * 500:(h + 1) * 500],
                            start=(d == 0), stop=(d == ND - 1))
                # 1/||p|| straight into partition 0 of the broadcast buffer
                nc.scalar.activation(
                    invpb[0:1, c0:c1].rearrange("p (h x) -> p h x", h=2),
                    psq[0:1, 2:4, :500], ACTF.Abs_reciprocal_sqrt)
                # broadcast to all 128 partitions (bf16 matmul) + copy out
                for h in range(2):
                    q0 = c0 + h * 500
                    nc.tensor.matmul(psq[:, h, :500], ones_r[:, :],
                                     invpb[0:1, q0:q0 + 500],
                                     start=True, stop=True)
                nc.scalar.copy(
                    invpb[:, c0:c1].rearrange("p (h x) -> p h x", h=2),
                    psq[:, 0:2, :500])
                # in-place normalize: pn = pn * invp  (bf16, per d-tile)
                for d in range(ND):
                    nc.vector.tensor_tensor(pn[:, d, c0:c1],
                                            pn[:, d, c0:c1],
                                            invpb[:, c0:c1], op=ALU.mult)

            # ---- main loop -------------------------------------------------
            for i in range(NB):
                rs = small_pool.tile([128, NG], F32, tag="rs")
                dq = dq_pool.tile([128, C], BF16, tag="dq")
                for g in range(NG):
                    pc = ps_c.tile([128, 4, 512], F32, tag="pc")
                    for k in range(4):
                        c0 = g * G1 + k * 500
                        for d in range(ND):
                            nc.tensor.matmul(
                                pc[:, k, :500],
                                fT[:, d, i * 128:(i + 1) * 128],
                                pn[:, d, c0:c0 + 500],
                                start=(d == 0), stop=(d == ND - 1))
                    # dist = sqrt(1 + G*invf); fused row-chunk sum
                    dv = dq[:, g * G1:(g + 1) * G1].rearrange(
                        "p (k x) -> p k x", k=4)
                    nc.scalar.activation(
                        dv, pc[:, :, :500], ACTF.Sqrt,
                        bias=1.0, scale=invf[:, i:i + 1],
                        accum_out=rs[:, g:g + 1])
                rsum = small_pool.tile([128, 1], F32, tag="rsum")
                bvec = small_pool.tile([128, 1], F32, tag="bvec")
                nc.vector.reduce_sum(rsum[:, :], rs[:, :],
                                     axis=mybir.AxisListType.X)
                nc.vector.tensor_scalar(bvec[:, :], rsum[:, :], cb[:, 1:2],
                                        None, op0=ALU.mult)
                for q in range(NOUT):
                    ob = ob_pool.tile([128, OUT], BF16, tag="ob")
                    nc.vector.tensor_scalar(ob[:, :],
                                            dq[:, q * OUT:(q + 1) * OUT],
                                            cb[:, 0:1], bvec[:, 0:1],
                                            op0=ALU.mult, op1=ALU.add)
                    nc.sync.dma_start(
                        o_h[i * 128:(i + 1) * 128, q * OUT:(q + 1) * OUT],
                        ob[:, :])

    nc.compile()
    return nc


_CACHE = {}


def _get_nc():
    if "nc" not in _CACHE:
        _CACHE["nc"] = build_nc()
    return _CACHE["nc"]


def make_in_maps(features, prototypes, distance_scale, temperature):
    f = np.asarray(features, dtype=np.float32)
    # negated so ACT's positive per-partition scale yields 1 - cos
    fneg = (-f).astype(ml_dtypes.bfloat16)
    pt = np.ascontiguousarray(
        np.asarray(prototypes, dtype=np.float32).T).astype(ml_dtypes.bfloat16)
    s = np.array([[np.float32(np.asarray(distance_scale).reshape(-1)[0]),
                   np.float32(np.asarray(temperature).reshape(-1)[0])]],
                 dtype=np.float32)
    in_maps = []
    for i in range(N_CORES):
        fi = fneg[i * BPC:(i + 1) * BPC]
        in_maps.append({
            "ft": np.ascontiguousarray(fi.T),
            "fb": np.ascontiguousarray(fi),
            "pt": pt,
            "s": s,
        })
    return in_maps


def run(features, prototypes, distance_scale, temperature, **kwargs):
    nc = _get_nc()
    in_maps = make_in_maps(features, prototypes, distance_scale, temperature)
    res = run_bass_kernel_spmd(nc, in_maps, core_ids=list(range(N_CORES)),
                               **kwargs)
    out = np.concatenate(
        [np.asarray(res.results[i]["o"]) for i in range(N_CORES)],
        axis=0).astype(np.float32)
    return out, res


def kernel(features, prototypes, distance_scale, temperature):
    out, _ = run(features, prototypes, distance_scale, temperature)
    return out


# revision 11
# speedup vs baseline: 1.3991x; 1.1503x over previous
"""DisMax loss first part: logits = -(|s|*d + mean_c(|s|*d)) / temp, where
d[b,c] = ||fn_b - pn_c|| / sqrt(2) = sqrt(1 - cos(f_b, p_c)) for l2-normalized rows.

Strategy: data-parallel over the batch across 8 NeuronCores. Each core:
  [1024, 512] features x [512, 10000] transposed prototypes -> [1024, 10000].
Both operands arrive host-transposed and host-cast to bf16 (layout/precision
prep only); all numerics run on device:
  - feature row norms: DVE square+accum on the batch-major copy, one ACT
    abs-rsqrt -> invf [128, 8]; features themselves stay RAW (negated on
    host) and invf folds into the main ACT as its per-partition scale;
  - prototype class norms: DVE/GPSIMD squares -> PE ones-matmul column
    sums -> chunked ACT abs-rsqrt (bf16 row) -> PE partition-broadcast ->
    in-place DVE normalize of the persistent bf16 pn operand;
  - main GEMM: bf16, fp32 PSUM, [128 x 500] chunks into 4-bank groups; ACT
    computes dist = sqrt(1 + G*invf) straight out of PSUM (G = -f.pn) with
    fused row-sum accumulation, writing bf16;
  - DVE applies out = dist*c0 + rowsum*c1 (c0 = -|scale|/temp,
    c1 = c0/10000) at 4x bf16 rate; 1.25 MB DMAs stream the bf16 result;
  - host upcasts the bf16 logits to f32 (within the 2e-2 tolerance).
"""

import sys
import types

for _p in ("/opt/trn_rl_repo", "/root/.axon_site"):
    if _p not in sys.path:
        sys.path.insert(0, _p)

# The NTFF profiling hook module is absent from this image's antenv package;
# inject the ctypes-based equivalent so trace=True works when requested.
if "antenv.axon_hooks" not in sys.modules:
    try:
        import trn_agent_boot.trn_boot as _tb

        _hook = _tb._ntff_profile_via_ctypes("/opt/axon/libaxon_pjrt.so")
        _m = types.ModuleType("antenv.axon_hooks")
        _m.get_axon_ntff_profile_hook = lambda: _hook
        sys.modules["antenv.axon_hooks"] = _m
    except Exception:
        pass

import ml_dtypes
import numpy as np

import concourse.bacc as bacc
import concourse.tile as tile
import concourse.mybir as mybir
from concourse.bass_utils import run_bass_kernel_spmd

F32 = mybir.dt.float32
BF16 = mybir.dt.bfloat16
ALU = mybir.AluOpType
ACTF = mybir.ActivationFunctionType

N_CORES = 8
B, C, D = 8192, 10000, 512
BPC = B // N_CORES          # 1024 batch rows per core
NB = BPC // 128             # 8 batch tiles
ND = D // 128               # 4 contraction tiles
PCH = 1000                  # prototype-prep chunk (columns)
NPCH = C // PCH             # 10
G1 = 1000                   # main-loop ACT group (2 psum banks)
NG = C // G1                # 10
OUT = 5000                  # output store chunk (1.25 MB bf16)
NOUT = C // OUT             # 2
# which prototype-prep square chunks run on gpsimd (rest on DVE)
SQ_GPS = {4, 9}


def build_nc():
    nc = bacc.Bacc("TRN2", target_bir_lowering=False, debug=False,
                   num_devices=N_CORES)
    ft_h = nc.dram_tensor("ft", [D, BPC], BF16, kind="ExternalInput")
    fb_h = nc.dram_tensor("fb", [BPC, D], BF16, kind="ExternalInput")
    pt_h = nc.dram_tensor("pt", [D, C], BF16, kind="ExternalInput")
    s_h = nc.dram_tensor("s", [1, 2], F32, kind="ExternalInput")
    o_h = nc.dram_tensor("o", [BPC, C], BF16, kind="ExternalOutput")

    from contextlib import ExitStack

    with tile.TileContext(nc) as tc:
        with ExitStack() as stack:
            ep = stack.enter_context
            const_pool = ep(tc.tile_pool(name="const", bufs=1))
            persist_pool = ep(tc.tile_pool(name="persist", bufs=1))
            fst_pool = ep(tc.tile_pool(name="fst", bufs=1))
            sq_pool = ep(tc.tile_pool(name="sq", bufs=3))
            dq_pool = ep(tc.tile_pool(name="dq", bufs=2))
            ob_pool = ep(tc.tile_pool(name="ob", bufs=2))
            small_pool = ep(tc.tile_pool(name="small", bufs=2))
            ps_c = ep(tc.tile_pool(name="ps_c", bufs=4, space="PSUM"))

            # persistent operands (pn is chunk-major: [p, chunk, d, col])
            pn = persist_pool.tile([128, NPCH, ND, PCH], BF16, tag="pn")
            fT = persist_pool.tile([128, ND, BPC], BF16, tag="fT")    # 8 KB/p
            invpb = persist_pool.tile([128, C], BF16, tag="invpb")    # 20 KB/p
            invf = persist_pool.tile([128, NB], F32, tag="invf")
            cb = persist_pool.tile([128, 2], F32, tag="cb")           # c0, c1

            ones_b = const_pool.tile([128, 1], BF16, tag="ones_b")
            nc.vector.memset(ones_b[:, :], 1.0)
            ones_r = const_pool.tile([1, 128], BF16, tag="ones_r")
            nc.vector.memset(ones_r[:, :], 1.0)
            ones_rf = const_pool.tile([1, 128], F32, tag="ones_rf")
            nc.vector.memset(ones_rf[:, :], 1.0)

            # ---- input DMAs (front-loaded) --------------------------------
            stile = const_pool.tile([1, 2], F32, tag="stile")
            nc.sync.dma_start(stile[:, :], s_h[:, :])
            ft_r = ft_h[:, :].rearrange("(t p) b -> p t b", p=128)
            nc.sync.dma_start(fT[:, :, :], ft_r)
            fb_r = fb_h[:, :].rearrange("(t p) d -> p t d", p=128)
            fst = fst_pool.tile([128, NB, D], BF16, tag="fst")
            nc.sync.dma_start(fst[:, :, :], fb_r)
            pt_r = pt_h[:, :].rearrange("(t p) c -> p t c", p=128)
            for c in range(NPCH):
                nc.sync.dma_start(pn[:, c, :, :],
                                  pt_r[:, :, c * PCH:(c + 1) * PCH])

            # ---- scalar params: c0 = -|ds|/temp, c1 = c0/C ----------------
            cv = const_pool.tile([1, 2], F32, tag="cvals")
            tmp = const_pool.tile([1, 2], F32, tag="scaltmp")
            nc.scalar.activation(tmp[:, 0:1], stile[:, 0:1], ACTF.Abs)
            nc.vector.reciprocal(tmp[:, 1:2], stile[:, 1:2])
            nc.vector.scalar_tensor_tensor(cv[:, 0:1], tmp[:, 0:1], -1.0,
                                           tmp[:, 1:2], op0=ALU.mult,
                                           op1=ALU.mult)
            nc.vector.tensor_scalar(cv[:, 1:2], cv[:, 0:1], 1.0 / C, None,
                                    op0=ALU.mult)
            ps_s = ps_c.tile([128, 2, 512], F32, tag="pc", name="ps_s")
            nc.tensor.matmul(ps_s[:, 0, :2], ones_rf[:, :], cv[:, :],
                             start=True, stop=True)
            nc.vector.tensor_copy(cb[:, :], ps_s[:, 0, :2])

            # ---- feature norms --------------------------------------------
            fss = small_pool.tile([128, NB], F32, tag="fss", bufs=1)
            for t in range(NB):
                fsq = sq_pool.tile([128, ND, PCH], BF16, tag="sq",
                                   name=f"fsq_{t}")
                fsqv = fsq[:, :, :].rearrange("p a b -> p (a b)")
                nc.vector.scalar_tensor_tensor(
                    fsqv[:, :D], fst[:, t, :], 1.0, fst[:, t, :],
                    op0=ALU.mult, op1=ALU.mult,
                    accum_out=fss[:, t:t + 1])
            nc.scalar.activation(invf[:, :], fss[:, :],
                                 ACTF.Abs_reciprocal_sqrt)

            # ---- prototype prep: norms + in-place normalize ----------------
            for c in range(NPCH):
                c0, c1 = c * PCH, (c + 1) * PCH
                sq = sq_pool.tile([128, ND * PCH], BF16, tag="sq",
                                  name=f"sq_{c}")
                pflat = pn[:, c, :, :].rearrange("p a b -> p (a b)")
                sqeng = nc.gpsimd if c in SQ_GPS else nc.vector
                sqeng.tensor_tensor(sq[:, :], pflat, pflat, op=ALU.mult)
                # column sums via ones-matmul into a 2-bank tile
                psq = ps_c.tile([128, 2, 512], F32, tag="pc",
                                name=f"psq_{c}")
                sqv = sq[:, :].rearrange("p (a b) -> p a b", a=ND)
                for h in range(2):
                    for d in range(ND):
                        nc.tensor.matmul(
                            psq[0:1, h, :500], ones_b[:, :],
                            sqv[:, d, h * 500:(h + 1) * 500],
                            start=(d == 0), stop=(d == ND - 1))
                # 1/||p|| straight into partition 0 of the broadcast buffer
                nc.scalar.activation(
                    invpb[0:1, c0:c1].rearrange("p (h x) -> p h x", h=2),
                    psq[0:1, :, :500], ACTF.Abs_reciprocal_sqrt)
                # broadcast to all 128 partitions (bf16 matmul) + copy out
                psb = ps_c.tile([128, 2, 512], F32, tag="pc",
                                name=f"psb_{c}")
                for h in range(2):
                    q0 = c0 + h * 500
                    nc.tensor.matmul(psb[:, h, :500], ones_r[:, :],
                                     invpb[0:1, q0:q0 + 500],
                                     start=True, stop=True)
                nc.scalar.copy(
                    invpb[:, c0:c1].rearrange("p (h x) -> p h x", h=2),
                    psb[:, :, :500])
                # in-place normalize: pn = pn * invp  (bf16, per d-tile)
                for d in range(ND):
                    nc.vector.tensor_tensor(pn[:, c, d, :],
                                            pn[:, c, d, :],
                                            invpb[:, c0:c1], op=ALU.mult)

            # ---- main loop -------------------------------------------------
            for i in range(NB):
                rs = small_pool.tile([128, NG], F32, tag="rs")
                dq = dq_pool.tile([128, C], BF16, tag="dq")
                for g in range(NG):
                    pc = ps_c.tile([128, 2, 512], F32, tag="pc")
                    for d in range(ND):
                        for h in range(2):
                            nc.tensor.matmul(
                                pc[:, h, :500],
                                fT[:, d, i * 128:(i + 1) * 128],
                                pn[:, g, d, h * 500:(h + 1) * 500],
                                start=(d == 0), stop=(d == ND - 1))
                    # dist = sqrt(1 + G*invf); fused row-chunk sum
                    dv = dq[:, g * G1:(g + 1) * G1].rearrange(
                        "p (h x) -> p h x", h=2)
                    nc.scalar.activation(
                        dv, pc[:, :, :500], ACTF.Sqrt,
                        bias=1.0, scale=invf[:, i:i + 1],
                        accum_out=rs[:, g:g + 1])
                rsum = small_pool.tile([128, 1], F32, tag="rsum")
                bvec = small_pool.tile([128, 1], F32, tag="bvec")
                nc.vector.reduce_sum(rsum[:, :], rs[:, :],
                                     axis=mybir.AxisListType.X)
                nc.vector.tensor_scalar(bvec[:, :], rsum[:, :], cb[:, 1:2],
                                        None, op0=ALU.mult)
                for q in range(NOUT):
                    ob = ob_pool.tile([128, OUT], BF16, tag="ob")
                    nc.vector.tensor_scalar(ob[:, :],
                                            dq[:, q * OUT:(q + 1) * OUT],
                                            cb[:, 0:1], bvec[:, 0:1],
                                            op0=ALU.mult, op1=ALU.add)
                    nc.sync.dma_start(
                        o_h[i * 128:(i + 1) * 128, q * OUT:(q + 1) * OUT],
                        ob[:, :])

    nc.compile()
    return nc


_CACHE = {}


def _get_nc():
    if "nc" not in _CACHE:
        _CACHE["nc"] = build_nc()
    return _CACHE["nc"]


def make_in_maps(features, prototypes, distance_scale, temperature):
    f = np.asarray(features, dtype=np.float32)
    # negated so ACT's positive per-partition scale yields 1 - cos
    fneg = (-f).astype(ml_dtypes.bfloat16)
    pt = np.ascontiguousarray(
        np.asarray(prototypes, dtype=np.float32).T).astype(ml_dtypes.bfloat16)
    s = np.array([[np.float32(np.asarray(distance_scale).reshape(-1)[0]),
                   np.float32(np.asarray(temperature).reshape(-1)[0])]],
                 dtype=np.float32)
    in_maps = []
    for i in range(N_CORES):
        fi = fneg[i * BPC:(i + 1) * BPC]
        in_maps.append({
            "ft": np.ascontiguousarray(fi.T),
            "fb": np.ascontiguousarray(fi),
            "pt": pt,
            "s": s,
        })
    return in_maps


def run(features, prototypes, distance_scale, temperature, **kwargs):
    nc = _get_nc()
    in_maps = make_in_maps(features, prototypes, distance_scale, temperature)
    res = run_bass_kernel_spmd(nc, in_maps, core_ids=list(range(N_CORES)),
                               **kwargs)
    out = np.concatenate(
        [np.asarray(res.results[i]["o"]) for i in range(N_CORES)],
        axis=0).astype(np.float32)
    return out, res


def kernel(features, prototypes, distance_scale, temperature):
    out, _ = run(features, prototypes, distance_scale, temperature)
    return out
